# revision 15
# baseline (speedup 1.0000x reference)
"""Bahdanau attention Trainium2 kernel.

Shapes (hardcoded): B=8, Te=512, Td=256, D=256, U=256.
Sharding: data-parallel over batch B across the 8 NeuronCores (1 batch el/core).

Per-core math:
    enc_projT[u,e] = (enc @ W1).T          # f32, PE
    dec_projT[u,t] = (dec @ W2).T + b1+b2  # f32, PE + DVE
    S[u,(t,e)] = enc_projT[u,e] + dec_projT[u,t]   # DVE tensor_scalar (2x mode)
    M = tanh(S) -> bf16                            # ACT, long instructions
    logits[t,e] = sum_u V[u]*M[u,(t,e)]            # PE: diagonal-V trick
    w = softmax_e(logits)  (no max-sub needed: |logits| <= ||V||_1 ~ 13)
    context[t,d] = sum_e w[t,e]*enc[e,d]           # PE (unnormalized, scaled after)

bV is ignored: softmax over e is exactly invariant to the scalar shift.
"""
import numpy as np
from contextlib import ExitStack

B, TE, TD, D, U = 8, 512, 256, 256, 256
NCORES = 8
SB = 32         # steps per sub-block, 16 from each t-chunk (tanh FD = SB*TE)
_CACHE = {}


def _build():
    import concourse.bass as bass
    from concourse import bacc
    import concourse.tile as tile
    import concourse.mybir as mybir
    from concourse.masks import make_identity

    F32 = mybir.dt.float32
    BF16 = mybir.dt.bfloat16
    Act = mybir.ActivationFunctionType

    nc = bacc.Bacc("TRN2", target_bir_lowering=False, debug=False)

    enc_d = nc.dram_tensor("encoder_outputs", [TE, D], F32, kind="ExternalInput").ap()
    dec_d = nc.dram_tensor("decoder_outputs", [TD, D], F32, kind="ExternalInput").ap()
    w1_d = nc.dram_tensor("W1", [D, U], F32, kind="ExternalInput").ap()
    b1_d = nc.dram_tensor("b1", [U], F32, kind="ExternalInput").ap()
    w2_d = nc.dram_tensor("W2", [D, U], F32, kind="ExternalInput").ap()
    b2_d = nc.dram_tensor("b2", [U], F32, kind="ExternalInput").ap()
    v_d = nc.dram_tensor("V", [U, 1], F32, kind="ExternalInput").ap()
    ctx_d = nc.dram_tensor("context", [TD, D], F32, kind="ExternalOutput").ap()
    attn_d = nc.dram_tensor("attn", [TD, TE], F32, kind="ExternalOutput").ap()

    with tile.TileContext(nc) as tc, ExitStack() as ctx:
        const = ctx.enter_context(tc.tile_pool(name="const", bufs=1))
        s_pool = ctx.enter_context(tc.tile_pool(name="s", bufs=2))
        m_pool = ctx.enter_context(tc.tile_pool(name="m", bufs=2))
        epi = ctx.enter_context(tc.tile_pool(name="epi", bufs=2))
        wt_pool = ctx.enter_context(tc.tile_pool(name="wt", bufs=10))
        small = ctx.enter_context(tc.tile_pool(name="small", bufs=4))
        ps_big = ctx.enter_context(tc.tile_pool(name="psb", bufs=2, space="PSUM"))
        ps_log = ctx.enter_context(tc.tile_pool(name="psl", bufs=1, space="PSUM"))
        ps_sm = ctx.enter_context(tc.tile_pool(name="pss", bufs=4, space="PSUM"))

        # ---- load inputs ----
        enc_sb = const.tile([128, 4, D], F32)       # [p, e-chunk, d]
        nc.sync.dma_start(enc_sb, enc_d.rearrange("(c p) d -> p c d", p=128))
        dec_sb = const.tile([128, 2, D], F32)
        nc.scalar.dma_start(dec_sb, dec_d.rearrange("(c p) d -> p c d", p=128))
        w1_sb = const.tile([128, 2, U], F32)        # [p(d), d-chunk, u]
        nc.sync.dma_start(w1_sb, w1_d.rearrange("(c p) u -> p c u", p=128))
        w2_sb = const.tile([128, 2, U], F32)
        nc.scalar.dma_start(w2_sb, w2_d.rearrange("(c p) u -> p c u", p=128))
        v_sb = const.tile([128, 2], F32)            # [p(u), u-chunk]
        nc.sync.dma_start(v_sb, v_d.rearrange("(c p) o -> p (c o)", p=128))
        b1_sb = const.tile([128, 2], F32)
        nc.sync.dma_start(b1_sb, b1_d.rearrange("(c p) -> p c", p=128))
        b2_sb = const.tile([128, 2], F32)
        nc.scalar.dma_start(b2_sb, b2_d.rearrange("(c p) -> p c", p=128))

        b_sb = const.tile([128, 2], F32)            # b1+b2 per u-chunk
        nc.vector.tensor_add(b_sb, b1_sb, b2_sb)

        ident = const.tile([128, 128], BF16)
        make_identity(nc, ident)

        # bf16 casts of the raw inputs (setup matmuls/transposes run in bf16)
        enc_bf = const.tile([128, 4, D], BF16)
        nc.vector.tensor_copy(enc_bf, enc_sb)
        dec_bf = const.tile([128, 2, D], BF16)
        nc.vector.tensor_copy(dec_bf, dec_sb)
        w1_bf = const.tile([128, 2, U], BF16)
        nc.vector.tensor_copy(w1_bf, w1_sb)
        w2_bf = const.tile([128, 2, U], BF16)
        nc.vector.tensor_copy(w2_bf, w2_sb)

        # ---- diagonal-V weights: vd_all[u, uc, j, i] = V[u+128*uc]*(i == j)
        vd_all = const.tile([128, 2, 32, 32], BF16)
        nc.gpsimd.memset(vd_all, 0.0)
        nc.gpsimd.affine_select(
            out=vd_all, in_=vd_all, compare_op=mybir.AluOpType.not_equal,
            fill=1.0, base=0, pattern=[[0, 2], [1, 32], [-1, 32]],
            channel_multiplier=0,
        )
        vd = [vd_all[:, 0], vd_all[:, 1]]
        for uc in range(2):
            nc.vector.tensor_scalar_mul(vd[uc], vd[uc], v_sb[:, uc:uc + 1])

        # ---- transpose enc, dec (bf16): encT[dc]=[p(d),e], decT[dc]=[p(d),t]
        encT, decT = [], []
        for dc in range(2):
            ps = ps_big.tile([128, TE], BF16, tag="ps_tr")
            for ec in range(4):
                nc.tensor.matmul(ps[:, ec * 128:(ec + 1) * 128],
                                 enc_bf[:, ec, dc * 128:(dc + 1) * 128],
                                 ident, is_transpose=True)
            t_ = const.tile([128, TE], BF16, tag=f"encT{dc}")
            nc.vector.tensor_copy(t_, ps)
            encT.append(t_)
        for dc in range(2):
            ps = ps_big.tile([128, TE], BF16, tag="ps_tr")
            for tc_ in range(2):
                nc.tensor.matmul(ps[:, tc_ * 128:(tc_ + 1) * 128],
                                 dec_bf[:, tc_, dc * 128:(dc + 1) * 128],
                                 ident, is_transpose=True)
            t_ = const.tile([128, TD], BF16, tag=f"decT{dc}")
            nc.vector.tensor_copy(t_, ps[:, :TD])
            decT.append(t_)

        # ---- projections (bf16 matmul, f32 psum)
        enc_projT, dec_projTb = [], []
        for uc in range(2):
            ps = ps_big.tile([128, TE], F32, tag="ps_tr")
            for dc in range(2):
                nc.tensor.matmul(ps, w1_bf[:, dc, uc * 128:(uc + 1) * 128],
                                 encT[dc], start=(dc == 0), stop=(dc == 1))
            t_ = const.tile([128, TE], BF16, tag=f"epj{uc}")
            nc.vector.tensor_copy(t_, ps)
            enc_projT.append(t_)
        for uc in range(2):
            ps = ps_sm.tile([128, TD], F32, tag="ps_sm")
            for dc in range(2):
                nc.tensor.matmul(ps, w2_bf[:, dc, uc * 128:(uc + 1) * 128],
                                 decT[dc], start=(dc == 0), stop=(dc == 1))
            t_ = const.tile([128, TD], F32, tag=f"dpj{uc}")
            nc.vector.tensor_scalar_add(t_, ps, b_sb[:, uc:uc + 1])
            dec_projTb.append(t_)

        # ---- main: 16 sub-blocks x 2 u-chunks; each sub-block covers 8
        # decoder steps from BOTH t-chunks so the logits matmuls alternate
        # between the two PSUM banks (avoids same-bank accumulate stalls).
        # A share of the broadcast-adds runs on GPSIMD to unload VectorE.
        # Unit sizes ramp up (quick first tanh -> ACT starts early) and down
        # (short final matmul chain -> epilogue starts early). Sum = 256.
        # (usb, banks): banks None = split half/half across both psum banks;
        # 0/1 = all steps go to that bank (used at the tail so bank0's
        # epilogue overlaps bank1's last compute).
        UNITS = ((8, None), (16, None), (24, None), (16, None), (32, None),
                 (32, None), (32, None), (32, None), (32, None),
                 (8, 0), (8, 1), (4, 0), (4, 1), (4, 0), (4, 1))
        assert sum(u for u, _ in UNITS) == 256
        lg = [ps_log.tile([128, TE], F32, tag=f"lg{i}", name=f"lg{i}")
              for i in range(2)]
        lt0 = [0, 0]  # per-bank decoder-step offset

        def jmap(usb, banks, j):
            if banks is None:
                half = usb // 2
                return (0 if j < half else 1), lt0[0] + (j % half)
            return banks, lt0[banks] + j

        for usb, banks in UNITS:
            for uc in range(2):
                s_t = s_pool.tile([128, usb, TE], BF16, tag="S")
                for j in range(usb):
                    tcb, lt = jmap(usb, banks, j)
                    col = tcb * 128 + lt
                    nc.vector.tensor_scalar_add(
                        s_t[:, j, :], enc_projT[uc],
                        dec_projTb[uc][:, col:col + 1])
                m_t = m_pool.tile([128, usb, TE], BF16, tag="M")
                nc.scalar.activation(
                    m_t.rearrange("p a b -> p (a b)"),
                    s_t.rearrange("p a b -> p (a b)"), Act.Tanh)
                half = usb // 2
                for jj in range(usb):
                    if banks is None:
                        j = (jj // 2) + half * (jj % 2)   # bank alternation
                    else:
                        j = jj
                    tcb, lt = jmap(usb, banks, j)
                    r = lt // 32
                    first = (lt % 32 == 0) and uc == 0
                    last = (lt % 32 == 31) and uc == 1
                    nc.tensor.matmul(lg[tcb][r * 32:(r + 1) * 32, :],
                                     vd[uc][:, lt % 32, :], m_t[:, j, :],
                                     start=first, stop=last,
                                     tile_position=(0, r * 32))
            if banks is None:
                lt0[0] += usb // 2
                lt0[1] += usb // 2
            else:
                lt0[banks] += usb
        assert lt0 == [128, 128]

        # ---- epilogues (one per t-chunk bank) ----
        for tcb in range(2):
            exp_t = epi.tile([128, TE], BF16, tag="exp")
            sums = small.tile([128, 1], F32, tag="sums")
            nc.scalar.activation(exp_t, lg[tcb], Act.Exp, accum_out=sums)
            rs = small.tile([128, 1], F32, tag="rs")
            nc.vector.reciprocal(rs, sums)
            w_t = epi.tile([128, TE], F32, tag="w")
            nc.vector.tensor_scalar_mul(w_t, exp_t, rs)
            nc.sync.dma_start(attn_d[tcb * 128:(tcb + 1) * 128, :], w_t)

            # context: transpose exp (bf16), matmul against enc, scale rows by rs
            ctx_ps = ps_sm.tile([128, D], F32, tag="ps_sm")
            for ec in range(4):
                tp = ps_sm.tile([128, 128], BF16, tag="ps_sm")
                nc.tensor.matmul(tp, exp_t[:, ec * 128:(ec + 1) * 128], ident,
                                 is_transpose=True)
                et = wt_pool.tile([128, 128], BF16, tag="expT")
                nc.vector.tensor_copy(et, tp)
                nc.tensor.matmul(ctx_ps, et, enc_bf[:, ec, :],
                                 start=(ec == 0), stop=(ec == 3))
            ctx_sb_t = epi.tile([128, D], F32, tag="ctx")
            nc.vector.tensor_scalar_mul(ctx_sb_t, ctx_ps, rs)
            nc.sync.dma_start(ctx_d[tcb * 128:(tcb + 1) * 128, :], ctx_sb_t)

    nc.compile()
    return nc


def get_nc():
    if "nc" not in _CACHE:
        _CACHE["nc"] = _build()
    return _CACHE["nc"]


def kernel(encoder_outputs, decoder_outputs, W1, b1, W2, b2, V, bV):
    from concourse.bass_utils import run_bass_kernel_spmd

    enc = np.asarray(encoder_outputs, dtype=np.float32)
    dec = np.asarray(decoder_outputs, dtype=np.float32)
    W1 = np.ascontiguousarray(np.asarray(W1, dtype=np.float32))
    W2 = np.ascontiguousarray(np.asarray(W2, dtype=np.float32))
    b1 = np.ascontiguousarray(np.asarray(b1, dtype=np.float32))
    b2 = np.ascontiguousarray(np.asarray(b2, dtype=np.float32))
    V = np.ascontiguousarray(np.asarray(V, dtype=np.float32))

    nc = get_nc()
    in_maps = [
        {
            "encoder_outputs": np.ascontiguousarray(enc[i]),
            "decoder_outputs": np.ascontiguousarray(dec[i]),
            "W1": W1, "b1": b1, "W2": W2, "b2": b2, "V": V,
        }
        for i in range(NCORES)
    ]
    res = run_bass_kernel_spmd(nc, in_maps, core_ids=list(range(NCORES)))
    ctx = np.stack([res.results[i]["context"] for i in range(NCORES)])
    attn = np.stack([res.results[i]["attn"] for i in range(NCORES)])
    return ctx.astype(np.float32), attn.reshape(B, TD, TE, 1).astype(np.float32)


# revision 16
# speedup vs baseline: 1.0170x; 1.0170x over previous
"""Bahdanau attention Trainium2 kernel.

Shapes (hardcoded): B=8, Te=512, Td=256, D=256, U=256.
Sharding: data-parallel over batch B across the 8 NeuronCores (1 batch el/core).

Per-core math:
    enc_projT[u,e] = (enc @ W1).T          # f32, PE
    dec_projT[u,t] = (dec @ W2).T + b1+b2  # f32, PE + DVE
    S[u,(t,e)] = enc_projT[u,e] + dec_projT[u,t]   # DVE tensor_scalar (2x mode)
    M = tanh(S) -> bf16                            # ACT, long instructions
    logits[t,e] = sum_u V[u]*M[u,(t,e)]            # PE: diagonal-V trick
    w = softmax_e(logits)  (no max-sub needed: |logits| <= ||V||_1 ~ 13)
    context[t,d] = sum_e w[t,e]*enc[e,d]           # PE (unnormalized, scaled after)

bV is ignored: softmax over e is exactly invariant to the scalar shift.
"""
import numpy as np
from contextlib import ExitStack

B, TE, TD, D, U = 8, 512, 256, 256, 256
NCORES = 8
SB = 32         # steps per sub-block, 16 from each t-chunk (tanh FD = SB*TE)
_CACHE = {}


def _build():
    import concourse.bass as bass
    from concourse import bacc
    import concourse.tile as tile
    import concourse.mybir as mybir
    from concourse.masks import make_identity

    F32 = mybir.dt.float32
    BF16 = mybir.dt.bfloat16
    Act = mybir.ActivationFunctionType

    nc = bacc.Bacc("TRN2", target_bir_lowering=False, debug=False)

    enc_d = nc.dram_tensor("encoder_outputs", [TE, D], F32, kind="ExternalInput").ap()
    dec_d = nc.dram_tensor("decoder_outputs", [TD, D], F32, kind="ExternalInput").ap()
    w1_d = nc.dram_tensor("W1", [D, U], F32, kind="ExternalInput").ap()
    b1_d = nc.dram_tensor("b1", [U], F32, kind="ExternalInput").ap()
    w2_d = nc.dram_tensor("W2", [D, U], F32, kind="ExternalInput").ap()
    b2_d = nc.dram_tensor("b2", [U], F32, kind="ExternalInput").ap()
    v_d = nc.dram_tensor("V", [U, 1], F32, kind="ExternalInput").ap()
    ctx_d = nc.dram_tensor("context", [TD, D], F32, kind="ExternalOutput").ap()
    attn_d = nc.dram_tensor("attn", [TD, TE], F32, kind="ExternalOutput").ap()

    with tile.TileContext(nc) as tc, ExitStack() as ctx:
        const = ctx.enter_context(tc.tile_pool(name="const", bufs=1))
        s_pool = ctx.enter_context(tc.tile_pool(name="s", bufs=2))
        m_pool = ctx.enter_context(tc.tile_pool(name="m", bufs=2))
        epi = ctx.enter_context(tc.tile_pool(name="epi", bufs=2))
        wt_pool = ctx.enter_context(tc.tile_pool(name="wt", bufs=10))
        small = ctx.enter_context(tc.tile_pool(name="small", bufs=4))
        ps_big = ctx.enter_context(tc.tile_pool(name="psb", bufs=2, space="PSUM"))
        ps_log = ctx.enter_context(tc.tile_pool(name="psl", bufs=1, space="PSUM"))
        ps_sm = ctx.enter_context(tc.tile_pool(name="pss", bufs=4, space="PSUM"))

        # ---- load inputs ----
        enc_sb = const.tile([128, 4, D], F32)       # [p, e-chunk, d]
        nc.sync.dma_start(enc_sb, enc_d.rearrange("(c p) d -> p c d", p=128))
        dec_sb = const.tile([128, 2, D], F32)
        nc.sync.dma_start(dec_sb, dec_d.rearrange("(c p) d -> p c d", p=128))
        w1_sb = const.tile([128, 2, U], F32)        # [p(d), d-chunk, u]
        nc.sync.dma_start(w1_sb, w1_d.rearrange("(c p) u -> p c u", p=128))
        w2_sb = const.tile([128, 2, U], F32)
        nc.sync.dma_start(w2_sb, w2_d.rearrange("(c p) u -> p c u", p=128))
        v_sb = const.tile([128, 2], F32)            # [p(u), u-chunk]
        nc.sync.dma_start(v_sb, v_d.rearrange("(c p) o -> p (c o)", p=128))
        b1_sb = const.tile([128, 2], F32)
        nc.sync.dma_start(b1_sb, b1_d.rearrange("(c p) -> p c", p=128))
        b2_sb = const.tile([128, 2], F32)
        nc.sync.dma_start(b2_sb, b2_d.rearrange("(c p) -> p c", p=128))

        b_sb = const.tile([128, 2], F32)            # b1+b2 per u-chunk
        nc.vector.tensor_add(b_sb, b1_sb, b2_sb)

        ident = const.tile([128, 128], BF16)
        make_identity(nc, ident)

        # bf16 casts of the raw inputs (setup matmuls/transposes run in bf16)
        enc_bf = const.tile([128, 4, D], BF16)
        nc.vector.tensor_copy(enc_bf, enc_sb)
        dec_bf = const.tile([128, 2, D], BF16)
        nc.vector.tensor_copy(dec_bf, dec_sb)
        w1_bf = const.tile([128, 2, U], BF16)
        nc.vector.tensor_copy(w1_bf, w1_sb)
        w2_bf = const.tile([128, 2, U], BF16)
        nc.vector.tensor_copy(w2_bf, w2_sb)

        # ---- diagonal-V weights: vd_all[u, uc, j, i] = V[u+128*uc]*(i == j)
        vd_all = const.tile([128, 2, 32, 32], BF16)
        nc.gpsimd.memset(vd_all, 0.0)
        nc.gpsimd.affine_select(
            out=vd_all, in_=vd_all, compare_op=mybir.AluOpType.not_equal,
            fill=1.0, base=0, pattern=[[0, 2], [1, 32], [-1, 32]],
            channel_multiplier=0,
        )
        vd = [vd_all[:, 0], vd_all[:, 1]]
        for uc in range(2):
            nc.vector.tensor_scalar_mul(vd[uc], vd[uc], v_sb[:, uc:uc + 1])

        # ---- transpose enc, dec (bf16): encT[dc]=[p(d),e], decT[dc]=[p(d),t]
        encT, decT = [], []
        for dc in range(2):
            ps = ps_big.tile([128, TE], BF16, tag="ps_tr")
            for ec in range(4):
                nc.tensor.matmul(ps[:, ec * 128:(ec + 1) * 128],
                                 enc_bf[:, ec, dc * 128:(dc + 1) * 128],
                                 ident, is_transpose=True)
            t_ = const.tile([128, TE], BF16, tag=f"encT{dc}")
            nc.vector.tensor_copy(t_, ps)
            encT.append(t_)
        for dc in range(2):
            ps = ps_big.tile([128, TE], BF16, tag="ps_tr")
            for tc_ in range(2):
                nc.tensor.matmul(ps[:, tc_ * 128:(tc_ + 1) * 128],
                                 dec_bf[:, tc_, dc * 128:(dc + 1) * 128],
                                 ident, is_transpose=True)
            t_ = const.tile([128, TD], BF16, tag=f"decT{dc}")
            nc.vector.tensor_copy(t_, ps[:, :TD])
            decT.append(t_)

        # ---- projections (bf16 matmul, f32 psum)
        enc_projT, dec_projTb = [], []
        for uc in range(2):
            ps = ps_big.tile([128, TE], F32, tag="ps_tr")
            for dc in range(2):
                nc.tensor.matmul(ps, w1_bf[:, dc, uc * 128:(uc + 1) * 128],
                                 encT[dc], start=(dc == 0), stop=(dc == 1))
            t_ = const.tile([128, TE], BF16, tag=f"epj{uc}")
            nc.vector.tensor_copy(t_, ps)
            enc_projT.append(t_)
        for uc in range(2):
            ps = ps_sm.tile([128, TD], F32, tag="ps_sm")
            for dc in range(2):
                nc.tensor.matmul(ps, w2_bf[:, dc, uc * 128:(uc + 1) * 128],
                                 decT[dc], start=(dc == 0), stop=(dc == 1))
            t_ = const.tile([128, TD], F32, tag=f"dpj{uc}")
            nc.vector.tensor_scalar_add(t_, ps, b_sb[:, uc:uc + 1])
            dec_projTb.append(t_)

        # ---- main: 16 sub-blocks x 2 u-chunks; each sub-block covers 8
        # decoder steps from BOTH t-chunks so the logits matmuls alternate
        # between the two PSUM banks (avoids same-bank accumulate stalls).
        # A share of the broadcast-adds runs on GPSIMD to unload VectorE.
        # Unit sizes ramp up (quick first tanh -> ACT starts early) and down
        # (short final matmul chain -> epilogue starts early). Sum = 256.
        # (usb, banks): banks None = split half/half across both psum banks;
        # 0/1 = all steps go to that bank (used at the tail so bank0's
        # epilogue overlaps bank1's last compute).
        UNITS = ((8, None), (8, None), (16, None), (32, None), (32, None),
                 (32, None), (32, None), (32, None), (32, None),
                 (16, 0), (16, 1))
        assert sum(u for u, _ in UNITS) == 256
        lg = [ps_log.tile([128, TE], F32, tag=f"lg{i}", name=f"lg{i}")
              for i in range(2)]
        lt0 = [0, 0]  # per-bank decoder-step offset

        def jmap(usb, banks, j):
            if banks is None:
                half = usb // 2
                return (0 if j < half else 1), lt0[0] + (j % half)
            return banks, lt0[banks] + j

        for usb, banks in UNITS:
            for uc in range(2):
                s_t = s_pool.tile([128, usb, TE], BF16, tag="S")
                for j in range(usb):
                    tcb, lt = jmap(usb, banks, j)
                    col = tcb * 128 + lt
                    nc.vector.tensor_scalar_add(
                        s_t[:, j, :], enc_projT[uc],
                        dec_projTb[uc][:, col:col + 1])
                m_t = m_pool.tile([128, usb, TE], BF16, tag="M")
                nc.scalar.activation(
                    m_t.rearrange("p a b -> p (a b)"),
                    s_t.rearrange("p a b -> p (a b)"), Act.Tanh)
                half = usb // 2
                for jj in range(usb):
                    if banks is None:
                        j = (jj // 2) + half * (jj % 2)   # bank alternation
                    else:
                        j = jj
                    tcb, lt = jmap(usb, banks, j)
                    r = lt // 32
                    first = (lt % 32 == 0) and uc == 0
                    last = (lt % 32 == 31) and uc == 1
                    nc.tensor.matmul(lg[tcb][r * 32:(r + 1) * 32, :],
                                     vd[uc][:, lt % 32, :], m_t[:, j, :],
                                     start=first, stop=last,
                                     tile_position=(0, r * 32))
            if banks is None:
                lt0[0] += usb // 2
                lt0[1] += usb // 2
            else:
                lt0[banks] += usb
        assert lt0 == [128, 128]

        # ---- epilogues (one per t-chunk bank) ----
        for tcb in range(2):
            exp_t = epi.tile([128, TE], BF16, tag="exp")
            sums = small.tile([128, 1], F32, tag="sums")
            nc.scalar.activation(exp_t, lg[tcb], Act.Exp, accum_out=sums)
            rs = small.tile([128, 1], F32, tag="rs")
            nc.vector.reciprocal(rs, sums)
            w_t = epi.tile([128, TE], F32, tag="w")
            nc.vector.tensor_scalar_mul(w_t, exp_t, rs)
            nc.sync.dma_start(attn_d[tcb * 128:(tcb + 1) * 128, :], w_t)

            # context: transpose exp (bf16), matmul against enc, scale rows by rs
            ctx_ps = ps_sm.tile([128, D], F32, tag="ps_sm")
            for ec in range(4):
                tp = ps_sm.tile([128, 128], BF16, tag="ps_sm")
                nc.tensor.matmul(tp, exp_t[:, ec * 128:(ec + 1) * 128], ident,
                                 is_transpose=True)
                et = wt_pool.tile([128, 128], BF16, tag="expT")
                nc.vector.tensor_copy(et, tp)
                nc.tensor.matmul(ctx_ps, et, enc_bf[:, ec, :],
                                 start=(ec == 0), stop=(ec == 3))
            ctx_sb_t = epi.tile([128, D], F32, tag="ctx")
            nc.vector.tensor_scalar_mul(ctx_sb_t, ctx_ps, rs)
            nc.sync.dma_start(ctx_d[tcb * 128:(tcb + 1) * 128, :], ctx_sb_t)

    nc.compile()
    return nc


def get_nc():
    if "nc" not in _CACHE:
        _CACHE["nc"] = _build()
    return _CACHE["nc"]


def kernel(encoder_outputs, decoder_outputs, W1, b1, W2, b2, V, bV):
    from concourse.bass_utils import run_bass_kernel_spmd

    enc = np.asarray(encoder_outputs, dtype=np.float32)
    dec = np.asarray(decoder_outputs, dtype=np.float32)
    W1 = np.ascontiguousarray(np.asarray(W1, dtype=np.float32))
    W2 = np.ascontiguousarray(np.asarray(W2, dtype=np.float32))
    b1 = np.ascontiguousarray(np.asarray(b1, dtype=np.float32))
    b2 = np.ascontiguousarray(np.asarray(b2, dtype=np.float32))
    V = np.ascontiguousarray(np.asarray(V, dtype=np.float32))

    nc = get_nc()
    in_maps = [
        {
            "encoder_outputs": np.ascontiguousarray(enc[i]),
            "decoder_outputs": np.ascontiguousarray(dec[i]),
            "W1": W1, "b1": b1, "W2": W2, "b2": b2, "V": V,
        }
        for i in range(NCORES)
    ]
    res = run_bass_kernel_spmd(nc, in_maps, core_ids=list(range(NCORES)))
    ctx = np.stack([res.results[i]["context"] for i in range(NCORES)])
    attn = np.stack([res.results[i]["attn"] for i in range(NCORES)])
    return ctx.astype(np.float32), attn.reshape(B, TD, TE, 1).astype(np.float32)


# revision 18
# speedup vs baseline: 1.0252x; 1.0080x over previous
"""Bahdanau attention Trainium2 kernel.

Shapes (hardcoded): B=8, Te=512, Td=256, D=256, U=256.
Sharding: data-parallel over batch B across the 8 NeuronCores (1 batch el/core).

Per-core math:
    enc_projT[u,e] = (enc @ W1).T          # f32, PE
    dec_projT[u,t] = (dec @ W2).T + b1+b2  # f32, PE + DVE
    S[u,(t,e)] = enc_projT[u,e] + dec_projT[u,t]   # DVE tensor_scalar (2x mode)
    M = tanh(S) -> bf16                            # ACT, long instructions
    logits[t,e] = sum_u V[u]*M[u,(t,e)]            # PE: diagonal-V trick
    w = softmax_e(logits)  (no max-sub needed: |logits| <= ||V||_1 ~ 13)
    context[t,d] = sum_e w[t,e]*enc[e,d]           # PE (unnormalized, scaled after)

bV is ignored: softmax over e is exactly invariant to the scalar shift.
"""
import numpy as np
from contextlib import ExitStack

B, TE, TD, D, U = 8, 512, 256, 256, 256
NCORES = 8
SB = 32         # steps per sub-block, 16 from each t-chunk (tanh FD = SB*TE)
_CACHE = {}


def _build():
    import concourse.bass as bass
    from concourse import bacc
    import concourse.tile as tile
    import concourse.mybir as mybir
    from concourse.masks import make_identity

    F32 = mybir.dt.float32
    BF16 = mybir.dt.bfloat16
    Act = mybir.ActivationFunctionType

    nc = bacc.Bacc("TRN2", target_bir_lowering=False, debug=False)

    enc_d = nc.dram_tensor("encoder_outputs", [TE, D], F32, kind="ExternalInput").ap()
    dec_d = nc.dram_tensor("decoder_outputs", [TD, D], F32, kind="ExternalInput").ap()
    w1_d = nc.dram_tensor("W1", [D, U], F32, kind="ExternalInput").ap()
    b1_d = nc.dram_tensor("b1", [U], F32, kind="ExternalInput").ap()
    w2_d = nc.dram_tensor("W2", [D, U], F32, kind="ExternalInput").ap()
    b2_d = nc.dram_tensor("b2", [U], F32, kind="ExternalInput").ap()
    v_d = nc.dram_tensor("V", [U, 1], F32, kind="ExternalInput").ap()
    ctx_d = nc.dram_tensor("context", [TD, D], F32, kind="ExternalOutput").ap()
    attn_d = nc.dram_tensor("attn", [TD, TE], F32, kind="ExternalOutput").ap()

    with tile.TileContext(nc) as tc, ExitStack() as ctx:
        const = ctx.enter_context(tc.tile_pool(name="const", bufs=1))
        s_pool = ctx.enter_context(tc.tile_pool(name="s", bufs=2))
        m_pool = ctx.enter_context(tc.tile_pool(name="m", bufs=2))
        epi = ctx.enter_context(tc.tile_pool(name="epi", bufs=2))
        wt_pool = ctx.enter_context(tc.tile_pool(name="wt", bufs=10))
        small = ctx.enter_context(tc.tile_pool(name="small", bufs=4))
        ps_big = ctx.enter_context(tc.tile_pool(name="psb", bufs=2, space="PSUM"))
        ps_log = ctx.enter_context(tc.tile_pool(name="psl", bufs=1, space="PSUM"))
        ps_sm = ctx.enter_context(tc.tile_pool(name="pss", bufs=4, space="PSUM"))

        # ---- load inputs ----
        enc_sb = const.tile([128, 4, D], F32)       # [p, e-chunk, d]
        nc.sync.dma_start(enc_sb, enc_d.rearrange("(c p) d -> p c d", p=128))
        w1_sb = const.tile([128, 2, U], F32)        # [p(d), d-chunk, u]
        nc.sync.dma_start(w1_sb, w1_d.rearrange("(c p) u -> p c u", p=128))
        dec_sb = const.tile([128, 2, D], F32)
        nc.sync.dma_start(dec_sb, dec_d.rearrange("(c p) d -> p c d", p=128))
        w2_sb = const.tile([128, 2, U], F32)
        nc.sync.dma_start(w2_sb, w2_d.rearrange("(c p) u -> p c u", p=128))
        v_sb = const.tile([128, 2], F32)            # [p(u), u-chunk]
        nc.sync.dma_start(v_sb, v_d.rearrange("(c p) o -> p (c o)", p=128))
        b1_sb = const.tile([128, 2], F32)
        nc.sync.dma_start(b1_sb, b1_d.rearrange("(c p) -> p c", p=128))
        b2_sb = const.tile([128, 2], F32)
        nc.sync.dma_start(b2_sb, b2_d.rearrange("(c p) -> p c", p=128))

        b_sb = const.tile([128, 2], F32)            # b1+b2 per u-chunk
        nc.vector.tensor_add(b_sb, b1_sb, b2_sb)

        ident = const.tile([128, 128], BF16)
        make_identity(nc, ident)

        # bf16 casts of the raw inputs (setup matmuls/transposes run in bf16)
        enc_bf = const.tile([128, 4, D], BF16)
        for ec in range(4):
            nc.vector.tensor_copy(enc_bf[:, ec], enc_sb[:, ec])
        dec_bf = const.tile([128, 2, D], BF16)
        nc.vector.tensor_copy(dec_bf, dec_sb)
        w1_bf = const.tile([128, 2, U], BF16)
        nc.vector.tensor_copy(w1_bf, w1_sb)
        w2_bf = const.tile([128, 2, U], BF16)
        nc.vector.tensor_copy(w2_bf, w2_sb)

        # ---- diagonal-V weights: vd_all[u, uc, j, i] = V[u+128*uc]*(i == j)
        vd_all = const.tile([128, 2, 32, 32], BF16)
        nc.gpsimd.memset(vd_all, 0.0)
        nc.gpsimd.affine_select(
            out=vd_all, in_=vd_all, compare_op=mybir.AluOpType.not_equal,
            fill=1.0, base=0, pattern=[[0, 2], [1, 32], [-1, 32]],
            channel_multiplier=0,
        )
        vd = [vd_all[:, 0], vd_all[:, 1]]
        for uc in range(2):
            nc.vector.tensor_scalar_mul(vd[uc], vd[uc], v_sb[:, uc:uc + 1])

        # ---- transpose enc, dec (bf16): encT[dc]=[p(d),e], decT[dc]=[p(d),t]
        encT, decT = [], []
        for dc in range(2):
            ps = ps_big.tile([128, TE], BF16, tag="ps_tr")
            for ec in range(4):
                nc.tensor.matmul(ps[:, ec * 128:(ec + 1) * 128],
                                 enc_bf[:, ec, dc * 128:(dc + 1) * 128],
                                 ident, is_transpose=True)
            t_ = const.tile([128, TE], BF16, tag=f"encT{dc}")
            nc.vector.tensor_copy(t_, ps)
            encT.append(t_)
        for dc in range(2):
            ps = ps_big.tile([128, TE], BF16, tag="ps_tr")
            for tc_ in range(2):
                nc.tensor.matmul(ps[:, tc_ * 128:(tc_ + 1) * 128],
                                 dec_bf[:, tc_, dc * 128:(dc + 1) * 128],
                                 ident, is_transpose=True)
            t_ = const.tile([128, TD], BF16, tag=f"decT{dc}")
            nc.vector.tensor_copy(t_, ps[:, :TD])
            decT.append(t_)

        # ---- projections (bf16 matmul, f32 psum)
        enc_projT, dec_projTb = [], []
        for uc in range(2):
            ps = ps_big.tile([128, TE], F32, tag="ps_tr")
            for dc in range(2):
                nc.tensor.matmul(ps, w1_bf[:, dc, uc * 128:(uc + 1) * 128],
                                 encT[dc], start=(dc == 0), stop=(dc == 1))
            t_ = const.tile([128, TE], BF16, tag=f"epj{uc}")
            nc.vector.tensor_copy(t_, ps)
            enc_projT.append(t_)
        for uc in range(2):
            ps = ps_sm.tile([128, TD], F32, tag="ps_sm")
            for dc in range(2):
                nc.tensor.matmul(ps, w2_bf[:, dc, uc * 128:(uc + 1) * 128],
                                 decT[dc], start=(dc == 0), stop=(dc == 1))
            t_ = const.tile([128, TD], F32, tag=f"dpj{uc}")
            nc.vector.tensor_scalar_add(t_, ps, b_sb[:, uc:uc + 1])
            dec_projTb.append(t_)

        # ---- main: 16 sub-blocks x 2 u-chunks; each sub-block covers 8
        # decoder steps from BOTH t-chunks so the logits matmuls alternate
        # between the two PSUM banks (avoids same-bank accumulate stalls).
        # A share of the broadcast-adds runs on GPSIMD to unload VectorE.
        # Unit sizes ramp up (quick first tanh -> ACT starts early) and down
        # (short final matmul chain -> epilogue starts early). Sum = 256.
        # (usb, banks): banks None = split half/half across both psum banks;
        # 0/1 = all steps go to that bank (used at the tail so bank0's
        # epilogue overlaps bank1's last compute).
        UNITS = ((4, None), (8, None), (20, None), (32, None), (32, None),
                 (32, None), (32, None), (32, None), (32, None),
                 (16, 0), (8, 1), (8, 1))
        assert sum(u for u, _ in UNITS) == 256
        lg = [ps_log.tile([128, TE], F32, tag=f"lg{i}", name=f"lg{i}")
              for i in range(2)]
        lt0 = [0, 0]  # per-bank decoder-step offset

        def jmap(usb, banks, j):
            if banks is None:
                half = usb // 2
                return (0 if j < half else 1), lt0[0] + (j % half)
            return banks, lt0[banks] + j

        for usb, banks in UNITS:
            for uc in range(2):
                s_t = s_pool.tile([128, usb, TE], BF16, tag="S")
                for j in range(usb):
                    tcb, lt = jmap(usb, banks, j)
                    col = tcb * 128 + lt
                    nc.vector.tensor_scalar_add(
                        s_t[:, j, :], enc_projT[uc],
                        dec_projTb[uc][:, col:col + 1])
                m_t = m_pool.tile([128, usb, TE], BF16, tag="M")
                nc.scalar.activation(
                    m_t.rearrange("p a b -> p (a b)"),
                    s_t.rearrange("p a b -> p (a b)"), Act.Tanh)
                half = usb // 2
                for jj in range(usb):
                    if banks is None:
                        j = (jj // 2) + half * (jj % 2)   # bank alternation
                    else:
                        j = jj
                    tcb, lt = jmap(usb, banks, j)
                    r = lt // 32
                    first = (lt % 32 == 0) and uc == 0
                    last = (lt % 32 == 31) and uc == 1
                    nc.tensor.matmul(lg[tcb][r * 32:(r + 1) * 32, :],
                                     vd[uc][:, lt % 32, :], m_t[:, j, :],
                                     start=first, stop=last,
                                     tile_position=(0, r * 32))
            if banks is None:
                lt0[0] += usb // 2
                lt0[1] += usb // 2
            else:
                lt0[banks] += usb
        assert lt0 == [128, 128]

        # ---- epilogues (one per t-chunk bank) ----
        for tcb in range(2):
            exp_t = epi.tile([128, TE], BF16, tag="exp")
            sums = small.tile([128, 1], F32, tag="sums")
            nc.scalar.activation(exp_t, lg[tcb], Act.Exp, accum_out=sums)
            rs = small.tile([128, 1], F32, tag="rs")
            nc.vector.reciprocal(rs, sums)
            w_t = epi.tile([128, TE], F32, tag="w")
            nc.vector.tensor_scalar_mul(w_t, exp_t, rs)
            nc.sync.dma_start(attn_d[tcb * 128:(tcb + 1) * 128, :], w_t)

            # context: transpose exp (bf16), matmul against enc, scale rows by rs
            ctx_ps = ps_sm.tile([128, D], F32, tag="ps_sm")
            for ec in range(4):
                tp = ps_sm.tile([128, 128], BF16, tag="ps_sm")
                nc.tensor.matmul(tp, exp_t[:, ec * 128:(ec + 1) * 128], ident,
                                 is_transpose=True)
                et = wt_pool.tile([128, 128], BF16, tag="expT")
                nc.vector.tensor_copy(et, tp)
                nc.tensor.matmul(ctx_ps, et, enc_bf[:, ec, :],
                                 start=(ec == 0), stop=(ec == 3))
            ctx_sb_t = epi.tile([128, D], F32, tag="ctx")
            nc.vector.tensor_scalar_mul(ctx_sb_t, ctx_ps, rs)
            nc.sync.dma_start(ctx_d[tcb * 128:(tcb + 1) * 128, :], ctx_sb_t)

    nc.compile()
    return nc


def get_nc():
    if "nc" not in _CACHE:
        _CACHE["nc"] = _build()
    return _CACHE["nc"]


def kernel(encoder_outputs, decoder_outputs, W1, b1, W2, b2, V, bV):
    from concourse.bass_utils import run_bass_kernel_spmd

    enc = np.asarray(encoder_outputs, dtype=np.float32)
    dec = np.asarray(decoder_outputs, dtype=np.float32)
    W1 = np.ascontiguousarray(np.asarray(W1, dtype=np.float32))
    W2 = np.ascontiguousarray(np.asarray(W2, dtype=np.float32))
    b1 = np.ascontiguousarray(np.asarray(b1, dtype=np.float32))
    b2 = np.ascontiguousarray(np.asarray(b2, dtype=np.float32))
    V = np.ascontiguousarray(np.asarray(V, dtype=np.float32))

    nc = get_nc()
    in_maps = [
        {
            "encoder_outputs": np.ascontiguousarray(enc[i]),
            "decoder_outputs": np.ascontiguousarray(dec[i]),
            "W1": W1, "b1": b1, "W2": W2, "b2": b2, "V": V,
        }
        for i in range(NCORES)
    ]
    res = run_bass_kernel_spmd(nc, in_maps, core_ids=list(range(NCORES)))
    ctx = np.stack([res.results[i]["context"] for i in range(NCORES)])
    attn = np.stack([res.results[i]["attn"] for i in range(NCORES)])
    return ctx.astype(np.float32), attn.reshape(B, TD, TE, 1).astype(np.float32)
